# revision 19
# baseline (speedup 1.0000x reference)
"""GraphTransformer (B=4, N=1024, H=8, D=256, L=4) on 8 TRN2 NeuronCores.

Sharding: core c -> (batch b = c//2, query-row half = c%2). Each core owns
R=512 query rows of one batch; k/v computed for all N=1024 rows locally from
an AllGather'd transposed hidden state. Between layers the local rows'
transposed hidden h2T is AllGather'd (bf16) within the 2-core pair.

Attention engine balance per (head-group, key-chunk):
  - scores: 4 concurrent row-tiled K=32 matmuls (tile_position=(32*hb, 0))
  - bias: heads 0-3 via DVE multiply by exp(bias); heads 4-7 via PE
    identity-matmul accumulate of raw bias into the score PSUM.
  - exp: one activation per 2-head [128,1024] f32 PSUM tile -> bf16 SBUF
  - attn@v + denominator: col-tiled matmuls (v/ones stationary, e moving)
    accumulating over key chunks; PSUM banks zeroed by an explicit
    zero-matmul so interleaved accumulation groups are well-defined.
  - normalize: reciprocal(den) + multiply; residuals are folded into the
    wa/w2 matmuls via identity stationaries; LN rstd = exp(-0.5*ln(var+eps))
    so only the exp/log activation table set is ever loaded.
"""

import sys

sys.path.insert(0, "/opt/trn_rl_repo")

import numpy as np
import ml_dtypes

B, N, H, D, L = 4, 1024, 8, 256, 4
SVD = 16
DK = D // H  # 32
EPS = 1e-6
R = 512  # local query rows per core
NCORES = 8
SCALE = 1.0 / float(np.sqrt(DK))

_CACHE = {}


def _patch_act_tables():
    """Restrict Exp/Ln to the natural_log_exp_and_others table set so the
    act-table-load pass emits a single load instead of thrashing between
    exp_and_others (Exp) and natural_log_exp_and_others (Ln) every LN."""
    from concourse import bacc as bacc_mod
    import functools

    if getattr(bacc_mod.get_activation_tables, "_ln_exp_patched", False):
        return
    orig = bacc_mod.get_activation_tables

    @functools.cache
    def patched(arch):
        from concourse import mybir

        AF = mybir.ActivationFunctionType
        out = {}
        for name, fns in orig(arch).items():
            if name != "natural_log_exp_and_others":
                fns = fns - {AF.Exp, AF.Ln}
            out[name] = fns
        return out

    patched._ln_exp_patched = True
    bacc_mod.get_activation_tables = patched


def _build_nc():
    import concourse.bass as bass
    from concourse import bacc, mybir, tile
    from concourse.masks import make_identity

    _patch_act_tables()

    f32 = mybir.dt.float32
    bf16 = mybir.dt.bfloat16
    AF = mybir.ActivationFunctionType

    nc = bacc.Bacc(
        "TRN2",
        target_bir_lowering=False,
        debug=False,
        num_devices=NCORES,
    )

    # ---- kernel I/O ----
    xT = nc.dram_tensor("xT", [D, N], bf16, kind="ExternalInput").ap()
    xTloc = nc.dram_tensor("xTloc", [D, R], bf16, kind="ExternalInput").ap()
    # heads 0-3: exp(bias); heads 4-7: raw bias. layout [jc, j, h(4), i]
    expbT = nc.dram_tensor("expbT", [8, 128, 4, R], bf16, kind="ExternalInput").ap()
    biasT = nc.dram_tensor("biasT", [8, 128, 4, R], bf16, kind="ExternalInput").ap()
    wts = {
        nm: nc.dram_tensor(nm, [L, 2, 128, D], bf16, kind="ExternalInput").ap()
        for nm in ["wq", "wk", "wv", "wa", "w1", "w2"]
    }
    out = nc.dram_tensor("out", [R, D], f32, kind="ExternalOutput").ap()

    groups = [[0, 1], [2, 3], [4, 5], [6, 7]]

    with tile.TileContext(nc) as tc:
        with (
            tc.tile_pool(name="const", bufs=1) as const,
            tc.tile_pool(name="state", bufs=2) as state,
            tc.tile_pool(name="work", bufs=2) as work,
            tc.tile_pool(name="epool", bufs=3) as epool,
            tc.tile_pool(name="gen", bufs=2, space="PSUM") as ps_gen,
            tc.tile_pool(name="score", bufs=2, space="PSUM") as ps_score,
            tc.tile_pool(name="acc", bufs=1, space="PSUM") as ps_acc,
            tc.tile_pool(name="dram", bufs=2, space="DRAM") as dram,
        ):
            # ---- constants ----
            idbf = const.tile([128, 128], bf16, tag="idbf")
            make_identity(nc, idbf)
            # [I | 0] and [0 | I] for folding residuals into matmuls
            idL = const.tile([128, 2, 128], bf16, tag="idL")
            nc.vector.memset(idL, 0.0)
            nc.vector.tensor_copy(out=idL[:, 0, :], in_=idbf)
            idR = const.tile([128, 2, 128], bf16, tag="idR")
            nc.vector.memset(idR, 0.0)
            nc.vector.tensor_copy(out=idR[:, 1, :], in_=idbf)
            ones32 = const.tile([128, 32], bf16, tag="ones32")
            nc.vector.memset(ones32, 1.0)
            zero128 = const.tile([128, 128], bf16, tag="zero128")
            nc.vector.memset(zero128, 0.0)
            dummy512 = const.tile([128, 512], bf16, tag="dummy512")
            nc.vector.memset(dummy512, 0.0)
            eps_t = const.tile([128, 1], f32, tag="eps")
            nc.vector.memset(eps_t, EPS)

            # ---- warm up the collective path: the first CC op pays ~45us of
            # one-time init; do it on a tiny dummy so it overlaps layer 0 ----
            warm_in = dram.tile([128, 4], bf16, tag="warm_in")
            warm_out = dram.tile([256, 4], bf16, tag="warm_out")
            warm_sb = const.tile([128, 4], bf16, tag="warm_sb")
            nc.vector.memset(warm_sb, 0.0)
            nc.sync.dma_start(out=warm_in, in_=warm_sb)
            nc.gpsimd.collective_compute(
                "AllGather",
                mybir.AluOpType.bypass,
                replica_groups=groups,
                ins=[warm_in.opt()],
                outs=[warm_out.opt()],
            )

            # ---- initial state (layer 0 inputs) ----
            hTloc = []
            for dt_ in range(2):
                t = state.tile([128, R], bf16, tag=f"hTl{dt_}")
                nc.sync.dma_start(out=t, in_=xTloc[dt_ * 128 : (dt_ + 1) * 128, :])
                hTloc.append(t)
            # full hidden state, split into the two half-collective column
            # groups: hT1 = queries {0-255, 512-767}, hT2 = {256-511, 768-1023}
            hT1, hT2 = [], []
            for dt_ in range(2):
                t1 = state.tile([128, 2, 256], bf16, tag=f"hT1_{dt_}", name="t1")
                nc.sync.dma_start(
                    out=t1[:, 0, :], in_=xT[dt_ * 128 : (dt_ + 1) * 128, 0:256]
                )
                nc.sync.dma_start(
                    out=t1[:, 1, :], in_=xT[dt_ * 128 : (dt_ + 1) * 128, 512:768]
                )
                hT1.append(t1)
                t2 = state.tile([128, 2, 256], bf16, tag=f"hT2_{dt_}", name="t2")
                nc.sync.dma_start(
                    out=t2[:, 0, :], in_=xT[dt_ * 128 : (dt_ + 1) * 128, 256:512]
                )
                nc.sync.dma_start(
                    out=t2[:, 1, :], in_=xT[dt_ * 128 : (dt_ + 1) * 128, 768:1024]
                )
                hT2.append(t2)

            def hT_col(dt_, jh4):
                """[128,256] slice of the hidden state for key cols
                [jh4*256,(jh4+1)*256); jh4 0,2 live in hT1, 1,3 in hT2."""
                src = hT1 if jh4 % 2 == 0 else hT2
                return src[dt_][:, jh4 // 2, :]

            def hT_rows(dt_, rt):
                """[128,128] slice for key rows [rt*128,(rt+1)*128)."""
                sl = hT_col(dt_, rt // 2)
                return sl[:, (rt % 2) * 128 : (rt % 2 + 1) * 128]

            # weights resident: [128, L, 2, D] per matrix
            w_sb = {}
            for nm in ["wq", "wk", "wv", "wa", "w1", "w2"]:
                t = const.tile([128, L, 2, D], bf16, tag=f"w_{nm}")
                nc.sync.dma_start(out=t, in_=wts[nm].rearrange("l c p d -> p l c d"))
                w_sb[nm] = t

            # bias tables resident
            expb_sb = []
            for jc in range(8):
                t = const.tile([128, 4, R], bf16, tag=f"expb{jc}")
                nc.sync.dma_start(out=t, in_=expbT[jc])
                expb_sb.append(t)
            bias_sb = []
            for jc in range(8):
                t = const.tile([128, 4, R], bf16, tag=f"bias{jc}")
                nc.sync.dma_start(out=t, in_=biasT[jc])
                bias_sb.append(t)

            def layer_norm_stats(z_ps):
                """First LN half: returns (mv, rstd) for later apply."""
                stats = work.tile([128, 6], f32, tag="ln_stats")
                nc.vector.bn_stats(out=stats, in_=z_ps)
                mv = work.tile([128, 2], f32, tag="ln_mv", bufs=3)
                nc.vector.bn_aggr(out=mv, in_=stats)
                # rstd = exp(-0.5 * ln(var + eps)) -- stays in the exp/log set
                lnv = work.tile([128, 1], f32, tag="ln_lnv")
                nc.scalar.activation(out=lnv, in_=mv[:, 1:2], func=AF.Ln, bias=eps_t)
                rstd = work.tile([128, 1], f32, tag="ln_rstd", bufs=3)
                nc.scalar.activation(out=rstd, in_=lnv, func=AF.Exp, scale=-0.5)
                return mv, rstd

            def layer_norm_apply(z_ps, mv, rstd, out_sb):
                nc.vector.tensor_scalar(
                    out=out_sb,
                    in0=z_ps,
                    scalar1=mv[:, 0:1],
                    scalar2=rstd,
                    op0=mybir.AluOpType.subtract,
                    op1=mybir.AluOpType.mult,
                )

            for t in range(L):
                # ---- qT [2 grp][128, R] bf16 (wq pre-scaled by 1/sqrt(dk)) ----
                qT = []
                for grp in range(2):
                    pst = ps_gen.tile([128, 512], f32, tag="gen")
                    for dt_ in range(2):
                        nc.tensor.matmul(
                            pst,
                            w_sb["wq"][:, t, dt_, grp * 128 : (grp + 1) * 128],
                            hTloc[dt_],
                            start=(dt_ == 0),
                            stop=(dt_ == 1),
                        )
                    sb = work.tile([128, R], bf16, tag=f"qT{grp}", bufs=2)
                    nc.vector.tensor_copy(out=sb, in_=pst)
                    qT.append(sb)

                # ---- kT [2 grp][128, N] bf16, in 256-col chunks ordered so the
                # chunks covered by the first half-collective come first ----
                kT = [
                    work.tile([128, N], bf16, tag=f"kT{grp}", name=f"kT{grp}", bufs=2)
                    for grp in range(2)
                ]
                for jh4 in [0, 2, 1, 3]:
                    for grp in range(2):
                        pst = ps_gen.tile([128, 256], f32, tag="gen", name="kps")
                        for dt_ in range(2):
                            nc.tensor.matmul(
                                pst,
                                w_sb["wk"][:, t, dt_, grp * 128 : (grp + 1) * 128],
                                hT_col(dt_, jh4),
                                start=(dt_ == 0),
                                stop=(dt_ == 1),
                            )
                        nc.vector.tensor_copy(
                            out=kT[grp][:, jh4 * 256 : (jh4 + 1) * 256], in_=pst
                        )

                # ---- v natural [j, d]: 4 tiles of 2 row-chunks; CC1 rows first
                v_sb = [None] * 4
                for rt2 in [0, 2, 1, 3]:
                    pst = ps_gen.tile([128, 2, 256], f32, tag="gen")
                    for k2 in range(2):
                        rt = rt2 * 2 + k2
                        for dt_ in range(2):
                            nc.tensor.matmul(
                                pst[:, k2, :],
                                hT_rows(dt_, rt),
                                w_sb["wv"][:, t, dt_, :],
                                start=(dt_ == 0),
                                stop=(dt_ == 1),
                            )
                    sb = work.tile([128, 2, 256], bf16, tag=f"v{rt2}", bufs=2)
                    nc.vector.tensor_copy(out=sb, in_=pst)
                    v_sb[rt2] = sb

                # ---- attention, one head-group (4 heads) at a time ----
                o_sbT = []
                for grp in range(2):
                    o_ps = ps_acc.tile([128, 512], f32, tag="o_ps", name="o_ps")
                    den_ps = ps_acc.tile([128, 512], f32, tag="den_ps", name="den_ps")
                    # zero the accumulator banks (sets has_written everywhere)
                    nc.tensor.matmul(
                        o_ps, zero128, dummy512, start=True, stop=False,
                        skip_group_check=True,
                    )
                    nc.tensor.matmul(
                        den_ps, zero128, dummy512, start=True, stop=False,
                        skip_group_check=True,
                    )
                    def emit_ov(e_pair, jc, last):
                        # band quads: all 4 col-bands of o, then of den, so
                        # the matmuls overlap in the PE array
                        for pr in range(2):
                            for k2 in range(2):
                                hb = pr * 2 + k2
                                dcol = grp * 128 + hb * 32
                                nc.tensor.matmul(
                                    o_ps[hb * 32 : (hb + 1) * 32, :],
                                    v_sb[jc // 2][:, jc % 2, dcol : dcol + 32],
                                    e_pair[pr][:, k2, :],
                                    start=False,
                                    stop=last and hb == 3,
                                    tile_position=(0, hb * 32),
                                    skip_group_check=True,
                                )
                        for pr in range(2):
                            for k2 in range(2):
                                hb = pr * 2 + k2
                                nc.tensor.matmul(
                                    den_ps[hb * 32 : (hb + 1) * 32, :],
                                    ones32,
                                    e_pair[pr][:, k2, :],
                                    start=False,
                                    stop=last and hb == 3,
                                    tile_position=(0, hb * 32),
                                    skip_group_check=True,
                                )

                    # software-pipelined: o/den matmuls for iteration jc are
                    # emitted after iteration jc+1's score matmuls, so the PE
                    # never has to wait for exp(jc) before starting scores.
                    pending = None
                    for jc in [0, 1, 4, 5, 2, 3, 6, 7]:
                        s_pair = []
                        for pr in range(2):
                            sp = ps_score.tile([128, 2, 512], f32, tag="score")
                            for k2 in range(2):
                                hb = pr * 2 + k2
                                nc.tensor.matmul(
                                    sp[:, k2, :],
                                    kT[grp][hb * 32 : (hb + 1) * 32,
                                            jc * 128 : (jc + 1) * 128],
                                    qT[grp][hb * 32 : (hb + 1) * 32, :],
                                    start=True,
                                    stop=(grp == 0),
                                    tile_position=(hb * 32, 0),
                                )
                            s_pair.append(sp)
                        if grp == 1:
                            # raw-bias accumulate via identity stationary
                            for pr in range(2):
                                for k2 in range(2):
                                    hb = pr * 2 + k2
                                    nc.tensor.matmul(
                                        s_pair[pr][:, k2, :],
                                        idbf,
                                        bias_sb[jc][:, hb, :],
                                        start=False,
                                        stop=True,
                                        tile_position=(0, 0),
                                    )
                        e_pair = []
                        for pr in range(2):
                            et = epool.tile([128, 2, 512], bf16, tag="e")
                            if grp == 0:
                                etmp = work.tile(
                                    [128, 2, 512], bf16, tag="etmp", bufs=2
                                )
                                nc.scalar.activation(
                                    out=etmp, in_=s_pair[pr], func=AF.Exp
                                )
                                nc.vector.tensor_mul(
                                    et, etmp, expb_sb[jc][:, 2 * pr : 2 * pr + 2, :]
                                )
                            else:
                                nc.scalar.activation(
                                    out=et, in_=s_pair[pr], func=AF.Exp
                                )
                            e_pair.append(et)
                        if pending is not None:
                            emit_ov(*pending, last=False)
                        pending = (e_pair, jc)
                    emit_ov(*pending, last=True)
                    # 1/den via exp(-ln(den)) on ScalarE (DVE reciprocal on
                    # [128,512] is ~6 cyc/elem); then o * recip on DVE.
                    d_ln = work.tile([128, 512], f32, tag="d_ln", bufs=2)
                    nc.scalar.activation(out=d_ln, in_=den_ps, func=AF.Ln)
                    d_rec = work.tile([128, 512], f32, tag="d_rec", bufs=2)
                    nc.scalar.activation(out=d_rec, in_=d_ln, func=AF.Exp, scale=-1.0)
                    osb = work.tile([128, 512], bf16, tag=f"oT{grp}", bufs=2)
                    nc.vector.tensor_mul(osb, o_ps, d_rec)
                    o_sbT.append(osb)

                # ---- z1 = o @ wa + h ; h1 = LN(z1); h1T ----
                h1T = [
                    work.tile([128, R], bf16, tag=f"h1T{dh}", name=f"h1T{dh}", bufs=2)
                    for dh in range(2)
                ]

                def ln_finish(pend, dest_T):
                    pst, mv, rstd, it2 = pend
                    cs2 = slice(it2 * 128, (it2 + 1) * 128)
                    zf2 = pst.rearrange("p a b -> p (a b)")
                    hbf = work.tile([128, D], bf16, tag="hbf", bufs=2)
                    layer_norm_apply(zf2, mv, rstd, hbf)
                    for dh in range(2):
                        # transposes borrow the (idle) score pool's banks
                        tp = ps_score.tile([128, 128], bf16, tag="score", name="tp")
                        nc.tensor.transpose(
                            tp, hbf[:, dh * 128 : (dh + 1) * 128], idbf
                        )
                        nc.vector.tensor_copy(out=dest_T[dh][:, cs2], in_=tp)
                    return hbf

                pend = None
                for it in range(4):
                    pst = ps_gen.tile([128, 2, 128], f32, tag="gen")
                    cs = slice(it * 128, (it + 1) * 128)
                    nc.tensor.matmul(
                        pst.rearrange("p a b -> p (a b)"),
                        o_sbT[0][:, cs], w_sb["wa"][:, t, 0, :],
                        start=True, stop=False,
                    )
                    nc.tensor.matmul(
                        pst.rearrange("p a b -> p (a b)"),
                        o_sbT[1][:, cs], w_sb["wa"][:, t, 1, :],
                        start=False, stop=False,
                    )
                    nc.tensor.matmul(
                        pst, hTloc[0][:, cs], idL, start=False, stop=False,
                    )
                    nc.tensor.matmul(
                        pst, hTloc[1][:, cs], idR, start=False, stop=True,
                    )
                    mv, rstd = layer_norm_stats(pst.rearrange("p a b -> p (a b)"))
                    if pend is not None:
                        ln_finish(pend, h1T)
                    pend = (pst, mv, rstd, it)
                ln_finish(pend, h1T)

                # ---- FFN: f1T = relu(W1^T h1T) ----
                f1T = []
                for grp in range(2):
                    pst = ps_gen.tile([128, 512], f32, tag="gen")
                    for dt_ in range(2):
                        nc.tensor.matmul(
                            pst,
                            w_sb["w1"][:, t, dt_, grp * 128 : (grp + 1) * 128],
                            h1T[dt_],
                            start=(dt_ == 0),
                            stop=(dt_ == 1),
                        )
                    sb = work.tile([128, R], bf16, tag=f"f1T{grp}", bufs=2)
                    nc.vector.tensor_scalar_max(sb, pst, 0.0)
                    f1T.append(sb)

                # ---- z2 = f1 @ W2 + h1 ; h2 = LN(z2) ----
                final = t == L - 1
                if not final:
                    hTloc_n = [
                        state.tile([128, R], bf16, tag=f"hTl{dh}", name=f"hTl{dh}")
                        for dh in range(2)
                    ]
                if not final:
                    # two half-collectives: rows it0/it1 then it2/it3, so the
                    # first half's exchange overlaps the second half's LN
                    cc_in = [
                        dram.tile([D, 256], bf16, tag=f"cc_in{hf}", name=f"ci{hf}")
                        for hf in range(2)
                    ]
                    cc_out = [
                        dram.tile([2 * D, 256], bf16, tag=f"cc_out{hf}", name=f"co{hf}")
                        for hf in range(2)
                    ]

                def ln2_finish(pend):
                    pst, mv, rstd, it2 = pend
                    cs2 = slice(it2 * 128, (it2 + 1) * 128)
                    zf2 = pst.rearrange("p a b -> p (a b)")
                    if final:
                        h2 = work.tile([128, D], f32, tag="h2f", bufs=2)
                        layer_norm_apply(zf2, mv, rstd, h2)
                        nc.sync.dma_start(out=out[cs2, :], in_=h2)
                        return
                    h2 = work.tile([128, D], bf16, tag="hbf", bufs=2)
                    layer_norm_apply(zf2, mv, rstd, h2)
                    hf, q = it2 // 2, it2 % 2
                    for dh in range(2):
                        tp = ps_score.tile(
                            [128, 128], bf16, tag="score", name="tp"
                        )
                        nc.tensor.transpose(
                            tp, h2[:, dh * 128 : (dh + 1) * 128], idbf
                        )
                        nc.vector.tensor_copy(out=hTloc_n[dh][:, cs2], in_=tp)
                        nc.sync.dma_start(
                            out=cc_in[hf][
                                dh * 128 : (dh + 1) * 128, q * 128 : (q + 1) * 128
                            ],
                            in_=hTloc_n[dh][:, cs2],
                        )
                    if q == 1:
                        nc.gpsimd.collective_compute(
                            "AllGather",
                            mybir.AluOpType.bypass,
                            replica_groups=groups,
                            ins=[cc_in[hf].opt()],
                            outs=[cc_out[hf].opt()],
                        )

                pend = None
                for it in range(4):
                    pst = ps_gen.tile([128, 2, 128], f32, tag="gen")
                    cs = slice(it * 128, (it + 1) * 128)
                    nc.tensor.matmul(
                        pst.rearrange("p a b -> p (a b)"),
                        f1T[0][:, cs], w_sb["w2"][:, t, 0, :],
                        start=True, stop=False,
                    )
                    nc.tensor.matmul(
                        pst.rearrange("p a b -> p (a b)"),
                        f1T[1][:, cs], w_sb["w2"][:, t, 1, :],
                        start=False, stop=False,
                    )
                    nc.tensor.matmul(
                        pst, h1T[0][:, cs], idL, start=False, stop=False,
                    )
                    nc.tensor.matmul(
                        pst, h1T[1][:, cs], idR, start=False, stop=True,
                    )
                    mv, rstd = layer_norm_stats(pst.rearrange("p a b -> p (a b)"))
                    if pend is not None:
                        ln2_finish(pend)
                    pend = (pst, mv, rstd, it)
                ln2_finish(pend)

                if not final:
                    # ---- rebuild hT1/hT2 from the two half-gathers ----
                    # cc_out[hf] rows: [pair-core0 d x 256q | pair-core1 d x 256q]
                    # hf=0 -> queries {0-255 (core0), 512-767 (core1)} = hT1
                    # hf=1 -> queries {256-511, 768-1023} = hT2
                    hT1_n, hT2_n = [], []
                    for hf in range(2):
                        for dt_ in range(2):
                            nt = state.tile(
                                [128, 2, 256], bf16,
                                tag=f"hT{hf + 1}_{dt_}", name="nt",
                            )
                            nc.sync.dma_start(
                                out=nt[:, 0, :],
                                in_=cc_out[hf][dt_ * 128 : (dt_ + 1) * 128, :],
                            )
                            nc.sync.dma_start(
                                out=nt[:, 1, :],
                                in_=cc_out[hf][D + dt_ * 128 : D + (dt_ + 1) * 128, :],
                            )
                            (hT1_n if hf == 0 else hT2_n).append(nt)
                    hT1, hT2 = hT1_n, hT2_n
                    hTloc = hTloc_n

    nc.compile()
    return nc


def _get_nc():
    if "nc" not in _CACHE:
        _CACHE["nc"] = _build_nc()
    return _CACHE["nc"]


def _host_prep(inputs):
    bf = ml_dtypes.bfloat16
    x = np.asarray(inputs["x"], np.float32)
    in_deg = np.asarray(inputs["in_degrees"]).astype(np.int64)
    out_deg = np.asarray(inputs["out_degrees"]).astype(np.int64)
    sp = np.asarray(inputs["spatial_pos"]).astype(np.int64)
    svd = np.asarray(inputs["svd_emb"], np.float32)

    pre = (
        np.asarray(inputs["in_deg_emb"], np.float32)[in_deg]
        + np.asarray(inputs["out_deg_emb"], np.float32)[out_deg]
    )
    pos = np.concatenate([svd[:, :SVD], -svd[:, SVD:]], axis=-1)
    pre = pre + pos @ np.asarray(inputs["W_svd"], np.float32) + np.asarray(
        inputs["b_svd"], np.float32
    )
    xp = x + pre[None]  # [B, N, D]

    # attn_bias[h, q, k] = spatial_emb[spatial_pos[q, k], h]
    bias_full = np.asarray(inputs["spatial_emb"], np.float32)[sp]  # [N(q), N(k), H]

    w_payload = {}
    for key, nm in [
        ("Wq", "wq"), ("Wk", "wk"), ("Wv", "wv"),
        ("Wa", "wa"), ("W1", "w1"), ("W2", "w2"),
    ]:
        w = np.asarray(inputs[key], np.float32)  # [L, D, D]
        if nm == "wq":
            w = w * SCALE
        w_payload[nm] = np.ascontiguousarray(w.reshape(L, 2, 128, D).astype(bf))

    in_maps = []
    for c in range(NCORES):
        b, half = c // 2, c % 2
        r0 = half * R
        xb = xp[b]  # [N, D] f32
        # [k, h, q_local] -> [jc, j, h, i]
        bloc = bias_full[r0 : r0 + R].transpose(1, 2, 0).reshape(8, 128, H, R)
        m = {
            "xT": np.ascontiguousarray(xb.T.astype(bf)),
            "xTloc": np.ascontiguousarray(xb[r0 : r0 + R].T.astype(bf)),
            "expbT": np.ascontiguousarray(np.exp(bloc[:, :, 0:4, :]).astype(bf)),
            "biasT": np.ascontiguousarray(bloc[:, :, 4:8, :].astype(bf)),
        }
        m.update(w_payload)
        in_maps.append(m)
    return in_maps


def kernel(**inputs):
    from concourse.bass_utils import run_bass_kernel_spmd

    nc = _get_nc()
    in_maps = _host_prep(inputs)
    res = run_bass_kernel_spmd(nc, in_maps, core_ids=list(range(NCORES)))
    out = np.empty((B, N, D), np.float32)
    for c in range(NCORES):
        b, half = c // 2, c % 2
        out[b, half * R : (half + 1) * R] = res.results[c]["out"]
    return out


if __name__ == "__main__":
    nc = _get_nc()
    print("compiled OK")


# revision 25
# speedup vs baseline: 1.0257x; 1.0257x over previous
"""GraphTransformer (B=4, N=1024, H=8, D=256, L=4) on 8 TRN2 NeuronCores.

Sharding: core c -> (batch b = c//2, query-row half = c%2). Each core owns
R=512 query rows of one batch; k/v computed for all N=1024 rows locally from
an AllGather'd transposed hidden state. Between layers the local rows'
transposed hidden h2T is AllGather'd (bf16) within the 2-core pair.

Attention engine balance per (head-group, key-chunk):
  - scores: 4 concurrent row-tiled K=32 matmuls (tile_position=(32*hb, 0))
  - bias: heads 0-3 via DVE multiply by exp(bias); heads 4-7 via PE
    identity-matmul accumulate of raw bias into the score PSUM.
  - exp: one activation per 2-head [128,1024] f32 PSUM tile -> bf16 SBUF
  - attn@v + denominator: col-tiled matmuls (v/ones stationary, e moving)
    accumulating over key chunks; PSUM banks zeroed by an explicit
    zero-matmul so interleaved accumulation groups are well-defined.
  - normalize: reciprocal(den) + multiply; residuals are folded into the
    wa/w2 matmuls via identity stationaries; LN rstd = exp(-0.5*ln(var+eps))
    so only the exp/log activation table set is ever loaded.
"""

import sys

sys.path.insert(0, "/opt/trn_rl_repo")

import numpy as np
import ml_dtypes

B, N, H, D, L = 4, 1024, 8, 256, 4
SVD = 16
DK = D // H  # 32
EPS = 1e-6
R = 512  # local query rows per core
NCORES = 8
SCALE = 1.0 / float(np.sqrt(DK))

_CACHE = {}


def _patch_act_tables():
    """Restrict Exp/Ln to the natural_log_exp_and_others table set so the
    act-table-load pass emits a single load instead of thrashing between
    exp_and_others (Exp) and natural_log_exp_and_others (Ln) every LN."""
    from concourse import bacc as bacc_mod
    import functools

    if getattr(bacc_mod.get_activation_tables, "_ln_exp_patched", False):
        return
    orig = bacc_mod.get_activation_tables

    @functools.cache
    def patched(arch):
        from concourse import mybir

        AF = mybir.ActivationFunctionType
        out = {}
        for name, fns in orig(arch).items():
            if name != "natural_log_exp_and_others":
                fns = fns - {AF.Exp, AF.Ln}
            out[name] = fns
        return out

    patched._ln_exp_patched = True
    bacc_mod.get_activation_tables = patched


def _build_nc():
    import concourse.bass as bass
    from concourse import bacc, mybir, tile
    from concourse.masks import make_identity

    _patch_act_tables()

    f32 = mybir.dt.float32
    bf16 = mybir.dt.bfloat16
    AF = mybir.ActivationFunctionType

    nc = bacc.Bacc(
        "TRN2",
        target_bir_lowering=False,
        debug=False,
        num_devices=NCORES,
    )

    # ---- kernel I/O ----
    xT = nc.dram_tensor("xT", [D, N], bf16, kind="ExternalInput").ap()
    xTloc = nc.dram_tensor("xTloc", [D, R], bf16, kind="ExternalInput").ap()
    # heads 0-3: exp(bias); heads 4-7: raw bias. layout [jc, j, h(4), i]
    expbT = nc.dram_tensor("expbT", [8, 128, 4, R], bf16, kind="ExternalInput").ap()
    biasT = nc.dram_tensor("biasT", [8, 128, 4, R], bf16, kind="ExternalInput").ap()
    wts = {
        nm: nc.dram_tensor(nm, [L, 2, 128, D], bf16, kind="ExternalInput").ap()
        for nm in ["wq", "wk", "wv", "wa", "w1", "w2"]
    }
    out = nc.dram_tensor("out", [R, D], f32, kind="ExternalOutput").ap()

    groups = [[0, 1], [2, 3], [4, 5], [6, 7]]

    with tile.TileContext(nc) as tc:
        with (
            tc.tile_pool(name="const", bufs=1) as const,
            tc.tile_pool(name="state", bufs=2) as state,
            tc.tile_pool(name="work", bufs=2) as work,
            tc.tile_pool(name="epool", bufs=3) as epool,
            tc.tile_pool(name="gen", bufs=2, space="PSUM") as ps_gen,
            tc.tile_pool(name="score", bufs=2, space="PSUM") as ps_score,
            tc.tile_pool(name="acc", bufs=1, space="PSUM") as ps_acc,
            tc.tile_pool(name="dram", bufs=2, space="DRAM") as dram,
        ):
            # ---- constants ----
            idbf = const.tile([128, 128], bf16, tag="idbf")
            make_identity(nc, idbf)
            # [I | 0] and [0 | I] for folding residuals into matmuls
            idL = const.tile([128, 2, 128], bf16, tag="idL")
            nc.vector.memset(idL, 0.0)
            nc.vector.tensor_copy(out=idL[:, 0, :], in_=idbf)
            idR = const.tile([128, 2, 128], bf16, tag="idR")
            nc.vector.memset(idR, 0.0)
            nc.vector.tensor_copy(out=idR[:, 1, :], in_=idbf)
            ones32 = const.tile([128, 32], bf16, tag="ones32")
            nc.vector.memset(ones32, 1.0)
            zero128 = const.tile([128, 128], bf16, tag="zero128")
            nc.vector.memset(zero128, 0.0)
            dummy512 = const.tile([128, 512], bf16, tag="dummy512")
            nc.vector.memset(dummy512, 0.0)
            eps_t = const.tile([128, 1], f32, tag="eps")
            nc.vector.memset(eps_t, EPS)

            # ---- warm up the collective path: the first CC op pays ~45us of
            # one-time init; do it on a tiny dummy so it overlaps layer 0 ----
            warm_in = dram.tile([128, 4], bf16, tag="warm_in")
            warm_out = dram.tile([256, 4], bf16, tag="warm_out")
            warm_sb = const.tile([128, 4], bf16, tag="warm_sb")
            nc.vector.memset(warm_sb, 0.0)
            nc.sync.dma_start(out=warm_in, in_=warm_sb)
            nc.gpsimd.collective_compute(
                "AllGather",
                mybir.AluOpType.bypass,
                replica_groups=groups,
                ins=[warm_in.opt()],
                outs=[warm_out.opt()],
            )

            # ---- initial state (layer 0 inputs) ----
            hTloc = []
            for dt_ in range(2):
                t = state.tile([128, R], bf16, tag=f"hTl{dt_}")
                nc.sync.dma_start(out=t, in_=xTloc[dt_ * 128 : (dt_ + 1) * 128, :])
                hTloc.append(t)
            # full hidden state, split into the two half-collective column
            # groups: hT1 = queries {0-255, 512-767}, hT2 = {256-511, 768-1023}
            hT1, hT2 = [], []
            for dt_ in range(2):
                t1 = state.tile([128, 2, 256], bf16, tag=f"hT1_{dt_}", name="t1")
                nc.sync.dma_start(
                    out=t1[:, 0, :], in_=xT[dt_ * 128 : (dt_ + 1) * 128, 0:256]
                )
                nc.sync.dma_start(
                    out=t1[:, 1, :], in_=xT[dt_ * 128 : (dt_ + 1) * 128, 512:768]
                )
                hT1.append(t1)
                t2 = state.tile([128, 2, 256], bf16, tag=f"hT2_{dt_}", name="t2")
                nc.sync.dma_start(
                    out=t2[:, 0, :], in_=xT[dt_ * 128 : (dt_ + 1) * 128, 256:512]
                )
                nc.sync.dma_start(
                    out=t2[:, 1, :], in_=xT[dt_ * 128 : (dt_ + 1) * 128, 768:1024]
                )
                hT2.append(t2)

            def hT_col(dt_, jh4):
                """[128,256] slice of the hidden state for key cols
                [jh4*256,(jh4+1)*256); jh4 0,2 live in hT1, 1,3 in hT2."""
                src = hT1 if jh4 % 2 == 0 else hT2
                return src[dt_][:, jh4 // 2, :]

            def hT_rows(dt_, rt):
                """[128,128] slice for key rows [rt*128,(rt+1)*128)."""
                sl = hT_col(dt_, rt // 2)
                return sl[:, (rt % 2) * 128 : (rt % 2 + 1) * 128]

            # weights resident: [128, L, 2, D] per matrix
            w_sb = {}
            for nm in ["wq", "wk", "wv", "wa", "w1", "w2"]:
                t = const.tile([128, L, 2, D], bf16, tag=f"w_{nm}")
                nc.sync.dma_start(out=t, in_=wts[nm].rearrange("l c p d -> p l c d"))
                w_sb[nm] = t

            # bias tables resident
            expb_sb = []
            for jc in range(8):
                t = const.tile([128, 4, R], bf16, tag=f"expb{jc}")
                nc.sync.dma_start(out=t, in_=expbT[jc])
                expb_sb.append(t)
            bias_sb = []
            for jc in range(8):
                t = const.tile([128, 4, R], bf16, tag=f"bias{jc}")
                nc.sync.dma_start(out=t, in_=biasT[jc])
                bias_sb.append(t)

            def layer_norm_stats(z_ps):
                """First LN half: returns (mv, rstd) for later apply."""
                stats = work.tile([128, 6], f32, tag="ln_stats")
                nc.vector.bn_stats(out=stats, in_=z_ps)
                mv = work.tile([128, 2], f32, tag="ln_mv", bufs=3)
                nc.vector.bn_aggr(out=mv, in_=stats)
                # rstd = exp(-0.5 * ln(var + eps)) -- stays in the exp/log set
                lnv = work.tile([128, 1], f32, tag="ln_lnv")
                nc.scalar.activation(out=lnv, in_=mv[:, 1:2], func=AF.Ln, bias=eps_t)
                rstd = work.tile([128, 1], f32, tag="ln_rstd", bufs=3)
                nc.scalar.activation(out=rstd, in_=lnv, func=AF.Exp, scale=-0.5)
                return mv, rstd

            def layer_norm_apply(z_ps, mv, rstd, out_sb):
                nc.vector.tensor_scalar(
                    out=out_sb,
                    in0=z_ps,
                    scalar1=mv[:, 0:1],
                    scalar2=rstd,
                    op0=mybir.AluOpType.subtract,
                    op1=mybir.AluOpType.mult,
                )

            for t in range(L):
                # ---- qT [2 grp][128, R] bf16 (wq pre-scaled by 1/sqrt(dk)) ----
                qT = []
                for grp in range(2):
                    pst = ps_gen.tile([128, 512], f32, tag="gen")
                    for dt_ in range(2):
                        nc.tensor.matmul(
                            pst,
                            w_sb["wq"][:, t, dt_, grp * 128 : (grp + 1) * 128],
                            hTloc[dt_],
                            start=(dt_ == 0),
                            stop=(dt_ == 1),
                        )
                    sb = work.tile([128, R], bf16, tag=f"qT{grp}", bufs=2)
                    nc.vector.tensor_copy(out=sb, in_=pst)
                    qT.append(sb)

                # ---- k/v, emitted per collective-half so attention on the
                # first half's chunks starts before the second half lands ----
                kT = [
                    work.tile([128, N], bf16, tag=f"kT{grp}", name=f"kT{grp}", bufs=2)
                    for grp in range(2)
                ]
                v_sb = [None] * 4

                def emit_kv(cc_half):
                    chunks = [0, 2] if cc_half == 0 else [1, 3]
                    for jh4 in chunks:
                        for grp in range(2):
                            pst = ps_gen.tile([128, 256], f32, tag="gen", name="kps")
                            for dt_ in range(2):
                                nc.tensor.matmul(
                                    pst,
                                    w_sb["wk"][:, t, dt_,
                                               grp * 128 : (grp + 1) * 128],
                                    hT_col(dt_, jh4),
                                    start=(dt_ == 0),
                                    stop=(dt_ == 1),
                                )
                            nc.vector.tensor_copy(
                                out=kT[grp][:, jh4 * 256 : (jh4 + 1) * 256], in_=pst
                            )
                    for rt2 in chunks:
                        pst = ps_gen.tile([128, 2, 256], f32, tag="gen", name="vps")
                        for k2 in range(2):
                            rt = rt2 * 2 + k2
                            for dt_ in range(2):
                                nc.tensor.matmul(
                                    pst[:, k2, :],
                                    hT_rows(dt_, rt),
                                    w_sb["wv"][:, t, dt_, :],
                                    start=(dt_ == 0),
                                    stop=(dt_ == 1),
                                )
                        sb = work.tile(
                            [128, 2, 256], bf16, tag=f"v{rt2}", name=f"v{rt2}",
                            bufs=2,
                        )
                        nc.vector.tensor_copy(out=sb, in_=pst)
                        v_sb[rt2] = sb

                emit_kv(0)

                # ---- attention, one head-group (4 heads) at a time ----
                o_sbT = []
                for grp in range(2):
                    o_ps = ps_acc.tile([128, 512], f32, tag="o_ps", name="o_ps")
                    den_ps = ps_acc.tile([128, 512], f32, tag="den_ps", name="den_ps")
                    # zero the accumulator banks (sets has_written everywhere)
                    nc.tensor.matmul(
                        o_ps, zero128, dummy512, start=True, stop=False,
                        skip_group_check=True,
                    )
                    nc.tensor.matmul(
                        den_ps, zero128, dummy512, start=True, stop=False,
                        skip_group_check=True,
                    )
                    def emit_ov(e_pair, jc, last):
                        # band quads: all 4 col-bands of o, then of den, so
                        # the matmuls overlap in the PE array
                        for pr in range(2):
                            for k2 in range(2):
                                hb = pr * 2 + k2
                                dcol = grp * 128 + hb * 32
                                nc.tensor.matmul(
                                    o_ps[hb * 32 : (hb + 1) * 32, :],
                                    v_sb[jc // 2][:, jc % 2, dcol : dcol + 32],
                                    e_pair[pr][:, k2, :],
                                    start=False,
                                    stop=last and hb == 3,
                                    tile_position=(0, hb * 32),
                                    skip_group_check=True,
                                )
                        for pr in range(2):
                            for k2 in range(2):
                                hb = pr * 2 + k2
                                nc.tensor.matmul(
                                    den_ps[hb * 32 : (hb + 1) * 32, :],
                                    ones32,
                                    e_pair[pr][:, k2, :],
                                    start=False,
                                    stop=last and hb == 3,
                                    tile_position=(0, hb * 32),
                                    skip_group_check=True,
                                )

                    # software-pipelined: o/den matmuls for iteration jc are
                    # emitted after iteration jc+1's score matmuls, so the PE
                    # never has to wait for exp(jc) before starting scores.
                    pending = None
                    for idx, jc in enumerate([0, 1, 4, 5, 2, 3, 6, 7]):
                        if grp == 0 and idx == 4:
                            # second collective-half's k/v, emitted here so the
                            # PE queue works on them while exp runs on jc 0-5
                            emit_kv(1)
                        s_pair = []
                        for pr in range(2):
                            sp = ps_score.tile([128, 2, 512], f32, tag="score")
                            for k2 in range(2):
                                hb = pr * 2 + k2
                                nc.tensor.matmul(
                                    sp[:, k2, :],
                                    kT[grp][hb * 32 : (hb + 1) * 32,
                                            jc * 128 : (jc + 1) * 128],
                                    qT[grp][hb * 32 : (hb + 1) * 32, :],
                                    start=True,
                                    stop=(grp == 0),
                                    tile_position=(hb * 32, 0),
                                )
                            s_pair.append(sp)
                        if grp == 1:
                            # raw-bias accumulate via identity stationary
                            for pr in range(2):
                                for k2 in range(2):
                                    hb = pr * 2 + k2
                                    nc.tensor.matmul(
                                        s_pair[pr][:, k2, :],
                                        idbf,
                                        bias_sb[jc][:, hb, :],
                                        start=False,
                                        stop=True,
                                        tile_position=(0, 0),
                                    )
                        e_pair = []
                        for pr in range(2):
                            et = epool.tile([128, 2, 512], bf16, tag="e")
                            if grp == 0:
                                etmp = work.tile(
                                    [128, 2, 512], bf16, tag="etmp", bufs=2
                                )
                                nc.scalar.activation(
                                    out=etmp, in_=s_pair[pr], func=AF.Exp
                                )
                                nc.vector.tensor_mul(
                                    et, etmp, expb_sb[jc][:, 2 * pr : 2 * pr + 2, :]
                                )
                            else:
                                nc.scalar.activation(
                                    out=et, in_=s_pair[pr], func=AF.Exp
                                )
                            e_pair.append(et)
                        if pending is not None:
                            emit_ov(*pending, last=False)
                        pending = (e_pair, jc)
                    emit_ov(*pending, last=True)
                    # 1/den: grp0 on DVE (hides under grp1's exp stream),
                    # grp1 via exp(-ln(den)) on ScalarE (fast, DVE recip is
                    # ~6 cyc/elem and would sit on the dense critical path).
                    d_rec = work.tile([128, 512], f32, tag="d_rec", bufs=2)
                    if grp == 0:
                        nc.vector.reciprocal(d_rec, den_ps)
                    else:
                        d_ln = work.tile([128, 512], f32, tag="d_ln", bufs=2)
                        nc.scalar.activation(out=d_ln, in_=den_ps, func=AF.Ln)
                        nc.scalar.activation(
                            out=d_rec, in_=d_ln, func=AF.Exp, scale=-1.0
                        )
                    osb = work.tile([128, 512], bf16, tag=f"oT{grp}", bufs=2)
                    nc.vector.tensor_mul(osb, o_ps, d_rec)
                    o_sbT.append(osb)

                # ---- z1 = o @ wa + h ; h1 = LN(z1); h1T ----
                h1T = [
                    work.tile([128, R], bf16, tag=f"h1T{dh}", name=f"h1T{dh}", bufs=2)
                    for dh in range(2)
                ]

                def ln_finish(pend, dest_T):
                    pst, mv, rstd, it2 = pend
                    cs2 = slice(it2 * 128, (it2 + 1) * 128)
                    zf2 = pst.rearrange("p a b -> p (a b)")
                    hbf = work.tile([128, D], bf16, tag="hbf", bufs=2)
                    layer_norm_apply(zf2, mv, rstd, hbf)
                    for dh in range(2):
                        # transposes borrow the (idle) score pool's banks
                        tp = ps_score.tile([128, 128], bf16, tag="score", name="tp")
                        nc.tensor.transpose(
                            tp, hbf[:, dh * 128 : (dh + 1) * 128], idbf
                        )
                        nc.vector.tensor_copy(out=dest_T[dh][:, cs2], in_=tp)
                    return hbf

                final = t == L - 1
                if not final:
                    hTloc_n = [
                        state.tile([128, R], bf16, tag=f"hTl{dh}", name=f"hTl{dh}")
                        for dh in range(2)
                    ]
                if not final:
                    # two half-collectives: rows it0/it1 then it2/it3, so the
                    # first half's exchange overlaps the second half's LN
                    cc_in = [
                        dram.tile([D, 256], bf16, tag=f"cc_in{hf}", name=f"ci{hf}")
                        for hf in range(2)
                    ]
                    cc_out = [
                        dram.tile([2 * D, 256], bf16, tag=f"cc_out{hf}", name=f"co{hf}")
                        for hf in range(2)
                    ]

                def ln2_finish(pend):
                    pst, mv, rstd, it2 = pend
                    cs2 = slice(it2 * 128, (it2 + 1) * 128)
                    zf2 = pst.rearrange("p a b -> p (a b)")
                    if final:
                        h2 = work.tile([128, D], f32, tag="h2f", bufs=2)
                        layer_norm_apply(zf2, mv, rstd, h2)
                        nc.sync.dma_start(out=out[cs2, :], in_=h2)
                        return
                    h2 = work.tile([128, D], bf16, tag="hbf", bufs=2)
                    layer_norm_apply(zf2, mv, rstd, h2)
                    hf, q = it2 // 2, it2 % 2
                    for dh in range(2):
                        tp = ps_score.tile(
                            [128, 128], bf16, tag="score", name="tp"
                        )
                        nc.tensor.transpose(
                            tp, h2[:, dh * 128 : (dh + 1) * 128], idbf
                        )
                        nc.vector.tensor_copy(out=hTloc_n[dh][:, cs2], in_=tp)
                        nc.sync.dma_start(
                            out=cc_in[hf][
                                dh * 128 : (dh + 1) * 128, q * 128 : (q + 1) * 128
                            ],
                            in_=hTloc_n[dh][:, cs2],
                        )
                    if q == 1:
                        nc.gpsimd.collective_compute(
                            "AllGather",
                            mybir.AluOpType.bypass,
                            replica_groups=groups,
                            ins=[cc_in[hf].opt()],
                            outs=[cc_out[hf].opt()],
                        )

                # ---- dense phase in query-halves: wa+LN1, FFN, w2+LN2 for
                # rows [half*256,(half+1)*256); each half's collective fires
                # as soon as its LN2 is done, overlapping the other half ----
                f1T = [
                    work.tile([128, R], bf16, tag=f"f1T{grp}", name=f"f1T{grp}",
                              bufs=2)
                    for grp in range(2)
                ]
                for half in range(2):
                    hs = slice(half * 256, (half + 1) * 256)
                    pend = None
                    for it in (2 * half, 2 * half + 1):
                        pst = ps_gen.tile([128, 2, 128], f32, tag="gen")
                        cs = slice(it * 128, (it + 1) * 128)
                        nc.tensor.matmul(
                            pst.rearrange("p a b -> p (a b)"),
                            o_sbT[0][:, cs], w_sb["wa"][:, t, 0, :],
                            start=True, stop=False,
                        )
                        nc.tensor.matmul(
                            pst.rearrange("p a b -> p (a b)"),
                            o_sbT[1][:, cs], w_sb["wa"][:, t, 1, :],
                            start=False, stop=False,
                        )
                        nc.tensor.matmul(
                            pst, hTloc[0][:, cs], idL, start=False, stop=False,
                        )
                        nc.tensor.matmul(
                            pst, hTloc[1][:, cs], idR, start=False, stop=True,
                        )
                        mv, rstd = layer_norm_stats(
                            pst.rearrange("p a b -> p (a b)")
                        )
                        if pend is not None:
                            ln_finish(pend, h1T)
                        pend = (pst, mv, rstd, it)
                    ln_finish(pend, h1T)

                    for grp in range(2):
                        pst = ps_gen.tile([128, 256], f32, tag="gen", name="f1ps")
                        for dt_ in range(2):
                            nc.tensor.matmul(
                                pst,
                                w_sb["w1"][:, t, dt_, grp * 128 : (grp + 1) * 128],
                                h1T[dt_][:, hs],
                                start=(dt_ == 0),
                                stop=(dt_ == 1),
                            )
                        nc.vector.tensor_scalar_max(f1T[grp][:, hs], pst, 0.0)

                    pend = None
                    for it in (2 * half, 2 * half + 1):
                        pst = ps_gen.tile([128, 2, 128], f32, tag="gen")
                        cs = slice(it * 128, (it + 1) * 128)
                        nc.tensor.matmul(
                            pst.rearrange("p a b -> p (a b)"),
                            f1T[0][:, cs], w_sb["w2"][:, t, 0, :],
                            start=True, stop=False,
                        )
                        nc.tensor.matmul(
                            pst.rearrange("p a b -> p (a b)"),
                            f1T[1][:, cs], w_sb["w2"][:, t, 1, :],
                            start=False, stop=False,
                        )
                        nc.tensor.matmul(
                            pst, h1T[0][:, cs], idL, start=False, stop=False,
                        )
                        nc.tensor.matmul(
                            pst, h1T[1][:, cs], idR, start=False, stop=True,
                        )
                        mv, rstd = layer_norm_stats(
                            pst.rearrange("p a b -> p (a b)")
                        )
                        if pend is not None:
                            ln2_finish(pend)
                        pend = (pst, mv, rstd, it)
                    ln2_finish(pend)

                if not final:
                    # keep the PE's HAM activity monitor warm across the
                    # collective wait so post-boundary matmuls run at 2.4 GHz
                    for _w in range(12):
                        wt = ps_score.tile(
                            [128, 2, 512], f32, tag="score", name="warm"
                        )
                        nc.tensor.matmul(
                            wt[:, 0, :], zero128, dummy512, start=True, stop=True
                        )

                if not final:
                    # ---- rebuild hT1/hT2 from the two half-gathers ----
                    # cc_out[hf] rows: [pair-core0 d x 256q | pair-core1 d x 256q]
                    # hf=0 -> queries {0-255 (core0), 512-767 (core1)} = hT1
                    # hf=1 -> queries {256-511, 768-1023} = hT2
                    hT1_n, hT2_n = [], []
                    for hf in range(2):
                        src = cc_out[hf].rearrange(
                            "(o d p) c -> d p o c", o=2, d=2
                        )
                        for dt_ in range(2):
                            nt = state.tile(
                                [128, 2, 256], bf16,
                                tag=f"hT{hf + 1}_{dt_}", name="nt",
                            )
                            nc.sync.dma_start(out=nt, in_=src[dt_])
                            (hT1_n if hf == 0 else hT2_n).append(nt)
                    hT1, hT2 = hT1_n, hT2_n
                    hTloc = hTloc_n

    nc.compile()
    return nc


def _get_nc():
    if "nc" not in _CACHE:
        _CACHE["nc"] = _build_nc()
    return _CACHE["nc"]


def _host_prep(inputs):
    bf = ml_dtypes.bfloat16
    x = np.asarray(inputs["x"], np.float32)
    in_deg = np.asarray(inputs["in_degrees"]).astype(np.int64)
    out_deg = np.asarray(inputs["out_degrees"]).astype(np.int64)
    sp = np.asarray(inputs["spatial_pos"]).astype(np.int64)
    svd = np.asarray(inputs["svd_emb"], np.float32)

    pre = (
        np.asarray(inputs["in_deg_emb"], np.float32)[in_deg]
        + np.asarray(inputs["out_deg_emb"], np.float32)[out_deg]
    )
    pos = np.concatenate([svd[:, :SVD], -svd[:, SVD:]], axis=-1)
    pre = pre + pos @ np.asarray(inputs["W_svd"], np.float32) + np.asarray(
        inputs["b_svd"], np.float32
    )
    xp = x + pre[None]  # [B, N, D]

    # attn_bias[h, q, k] = spatial_emb[spatial_pos[q, k], h]
    bias_full = np.asarray(inputs["spatial_emb"], np.float32)[sp]  # [N(q), N(k), H]

    w_payload = {}
    for key, nm in [
        ("Wq", "wq"), ("Wk", "wk"), ("Wv", "wv"),
        ("Wa", "wa"), ("W1", "w1"), ("W2", "w2"),
    ]:
        w = np.asarray(inputs[key], np.float32)  # [L, D, D]
        if nm == "wq":
            w = w * SCALE
        w_payload[nm] = np.ascontiguousarray(w.reshape(L, 2, 128, D).astype(bf))

    in_maps = []
    for c in range(NCORES):
        b, half = c // 2, c % 2
        r0 = half * R
        xb = xp[b]  # [N, D] f32
        # [k, h, q_local] -> [jc, j, h, i]
        bloc = bias_full[r0 : r0 + R].transpose(1, 2, 0).reshape(8, 128, H, R)
        m = {
            "xT": np.ascontiguousarray(xb.T.astype(bf)),
            "xTloc": np.ascontiguousarray(xb[r0 : r0 + R].T.astype(bf)),
            "expbT": np.ascontiguousarray(np.exp(bloc[:, :, 0:4, :]).astype(bf)),
            "biasT": np.ascontiguousarray(bloc[:, :, 4:8, :].astype(bf)),
        }
        m.update(w_payload)
        in_maps.append(m)
    return in_maps


def kernel(**inputs):
    from concourse.bass_utils import run_bass_kernel_spmd

    nc = _get_nc()
    in_maps = _host_prep(inputs)
    res = run_bass_kernel_spmd(nc, in_maps, core_ids=list(range(NCORES)))
    out = np.empty((B, N, D), np.float32)
    for c in range(NCORES):
        b, half = c // 2, c % 2
        out[b, half * R : (half + 1) * R] = res.results[c]["out"]
    return out


if __name__ == "__main__":
    nc = _get_nc()
    print("compiled OK")
